# revision 1
# baseline (speedup 1.0000x reference)
"""Trainium2 Bass kernel for nn_LipschitzNet (8-core SPMD, batch-sharded).

Math (reference, with beta=0.75, gamma=0.01, dt=1e-3):
    A = M_A - 0.5*M_A.T - 0.01*I        W = M_W - 0.5*M_W.T - 0.01*I
    Z[t,d,h] = sum_b x[b,t,d] * E_w[h,b] + E_b[h]
    h_{t+1} = h_t + dt*(h_t @ A) + dt*tanh(h_t @ W + Z[t])
    out = h_T @ D_w.T + D_b

Rescaled recurrence used on device (H = h/dt, exact in infinite precision):
    H_{t+1} = H_t + dt*(H_t @ A) + tanh(dt*(H_t @ W) + Z[t])
    out = dt * (H_T @ D_w.T) + D_b

Sharding: batch rows (= the d index of x via the module's transpose-based
math) are split 16 per core; each core runs the full sequential scan on its
shard with replicated weights and writes its [16, 24] slice of the output.

Precision scheme: the identity path (H_t + ...) is an exact fp32 add; the
two weight streams run in fp8-e4m3 DoubleRow (2 MACs/cell/cycle) with the
weights pre-scaled by 2^13 and the dt/2^13 descale folded into the DVE
combine constants. Both weight contributions are dt-damped, so the fp8
quantization of weights and state perturbs the output by only ~4e-4
relative (numpy-emulated). Z and the final linear run in float32r.

State layout: G8 = H.T as 4 fp8 [128, 2, 16] k-pair tiles (DoubleRow's
virtual-256 contraction); each step's new H (batch-major fp32, in SBUF)
is re-transposed on the PE and converted to fp8 by DVE copies.
"""
import numpy as np

import concourse.bass as bass
import concourse.tile as tile
from concourse import bacc, mybir
from concourse.bass_utils import run_bass_kernel_spmd
from concourse.masks import make_identity

FP32 = mybir.dt.float32
FP32R = mybir.dt.float32r
FP8 = mybir.dt.float8e4
DR = mybir.MatmulPerfMode.DoubleRow
AF = mybir.ActivationFunctionType
ALU = mybir.AluOpType

HID = 1024
B = 128
T = 512
OUT = 24
DT = 0.001
NCORES = 8
BS = B // NCORES  # 16 batch rows per core
KT = HID // 128  # 8 k-tiles
KP = KT // 2  # 4 DoubleRow k-pairs
S8 = 8192.0  # fp8 weight pre-scale (2^13); descale folded into DVE consts
C8 = DT / S8


def _scaled_identity(nc, ap, c):
    nc.gpsimd.memset(ap, 0.0)
    nc.gpsimd.affine_select(
        out=ap,
        in_=ap,
        compare_op=ALU.not_equal,
        fill=c,
        base=0,
        pattern=[[-1, ap.shape[0]]],
        channel_multiplier=1,
    )


def build(t_steps=T, trace_sim=False):
    from contextlib import ExitStack

    assert t_steps % 8 == 0
    nc = bacc.Bacc("TRN2")
    xs = nc.dram_tensor("xs", [B, t_steps, BS], FP32, kind="ExternalInput")
    MA = nc.dram_tensor("MA", [HID, HID], FP32, kind="ExternalInput")
    MW = nc.dram_tensor("MW", [HID, HID], FP32, kind="ExternalInput")
    Ewt = nc.dram_tensor("Ewt", [B, HID], FP32, kind="ExternalInput")
    Ebb = nc.dram_tensor("Ebb", [B, HID], FP32, kind="ExternalInput")
    Dwt = nc.dram_tensor("Dwt", [HID, OUT], FP32, kind="ExternalInput")
    Dbb = nc.dram_tensor("Dbb", [B, OUT], FP32, kind="ExternalInput")
    out = nc.dram_tensor("out", [BS, OUT], FP32, kind="ExternalOutput")
    Zd = nc.dram_tensor("Zd", [t_steps // 8, B, HID], FP32R)

    with tile.TileContext(nc, trace_sim=trace_sim) as tc, ExitStack() as ctx:
        consts = ctx.enter_context(tc.tile_pool(name="consts", bufs=1))
        ident = consts.tile([128, 128], FP32)
        make_identity(nc, ident[:])
        identG = consts.tile([128, 128], FP32)  # -0.01 * S8 * I (gamma term)
        _scaled_identity(nc, identG[:], -0.01 * S8)
        identR = consts.tile([128, 128], FP32R)  # fp32r identity (z-select)
        nc.vector.tensor_copy(identR[:], ident[:])
        Ebb_sb = consts.tile([128, HID], FP32)
        nc.sync.dma_start(Ebb_sb[:], Ebb[:])
        Dbb_sb = consts.tile([128, OUT], FP32)
        nc.sync.dma_start(Dbb_sb[:], Dbb[:])
        Ewt_r = consts.tile([128, HID], FP32R)
        nc.gpsimd.dma_start(Ewt_r[:], Ewt[:])  # cast fp32 -> fp32r
        Dwt_r = consts.tile([128, KT * OUT], FP32R)
        nc.gpsimd.dma_start(Dwt_r[:], Dwt[:].rearrange("(k p) o -> p k o", p=128))
        # fp8 weight streams, [128, kpair, row, n] (DoubleRow layout)
        A8 = consts.tile([128, KP, 2, HID], FP8)
        W8 = consts.tile([128, KP, 2, HID], FP8)

        # ---- weight prep: X8 = S8 * (M - 0.5*M.T - 0.01*I) in fp8 ----
        with (
            tc.tile_pool(name="prep", bufs=1) as prep,
            tc.tile_pool(name="prep_ps", bufs=2, space="PSUM") as pps,
        ):
            for M, dst in ((MA, A8), (MW, W8)):
                stage = prep.tile([128, KT * HID], FP32, tag="stage")
                nc.sync.dma_start(
                    stage[:], M[:].rearrange("(k p) n -> p k n", p=128)
                )
                stageT = prep.tile([128, KT * HID], FP32, tag="stageT")
                for k in range(KT):
                    for j in range(KT):
                        trp = pps.tile([128, 128], FP32)
                        nc.tensor.transpose(
                            trp[:],
                            stage[:, j * HID + 128 * k : j * HID + 128 * (k + 1)],
                            ident[:],
                        )
                        nc.vector.tensor_copy(
                            stageT[:, k * HID + 128 * j : k * HID + 128 * (j + 1)],
                            trp[:],
                        )
                t1 = prep.tile([128, KT * HID], FP32, tag="t1")
                nc.vector.scalar_tensor_tensor(
                    t1[:], stageT[:], -0.5, stage[:], ALU.mult, ALU.add
                )
                for k in range(KT):
                    kp, r = divmod(k, 2)
                    nc.vector.tensor_scalar_mul(
                        dst[:, kp, r, :], t1[:, k * HID : (k + 1) * HID], S8
                    )
                    # diagonal block gets the -0.01*I correction
                    nc.vector.scalar_tensor_tensor(
                        dst[:, kp, r, 128 * k : 128 * (k + 1)],
                        t1[:, k * HID + 128 * k : k * HID + 128 * (k + 1)],
                        S8,
                        identG[:],
                        ALU.mult,
                        ALU.add,
                    )

        # ---- Z = (x^T E^T + E_b)/C8, fp32r octs [128(t,d), 1024] ----
        # Pre-scaled by 1/C8 so the scan can accumulate z into the W-psum
        # via an identity-selector matmul and tanh directly with scale=C8.
        Ebb_s = consts.tile([128, HID], FP32)
        nc.vector.tensor_scalar_mul(Ebb_s[:], Ebb_sb[:], 1.0 / C8)
        with (
            tc.tile_pool(name="zx", bufs=3) as zx,
            tc.tile_pool(name="zstage", bufs=3) as zs,
            tc.tile_pool(name="zps", bufs=2, space="PSUM") as zp,
        ):
            for o in range(t_steps // 8):
                xr = zx.tile([128, 128], FP32R)
                nc.gpsimd.dma_start(xr[:], xs[:, 8 * o : 8 * o + 8, :])
                ps = zp.tile([128, HID], FP32)
                for h in range(2):
                    nc.tensor.matmul(
                        ps[:, 512 * h : 512 * (h + 1)],
                        xr[:],
                        Ewt_r[:, 512 * h : 512 * (h + 1)],
                        start=True,
                        stop=True,
                    )
                zst = zs.tile([128, HID], FP32R)
                nc.vector.scalar_tensor_tensor(
                    zst[:], ps[:], 1.0 / C8, Ebb_s[:], ALU.mult, ALU.add
                )
                nc.sync.dma_start(Zd[o], zst[:])

        # ---- the sequential scan ----
        # fp8 DoubleRow for the two weight streams; z is injected into the
        # W-psum by an fp32r identity-selector matmul from full-width oct
        # tiles (tanh then reads the psum with scale=C8); the identity path
        # H' = H + tmp is an exact fp32 add on gpsimd, off the DVE path.
        with (
            tc.tile_pool(name="g", bufs=2) as gp,
            tc.tile_pool(name="zt", bufs=2) as ztp,
            tc.tile_pool(name="u", bufs=2) as up,
            tc.tile_pool(name="s", bufs=2) as spp,
            tc.tile_pool(name="hn", bufs=3) as hnp,
        ):
            G8 = []
            g0f = gp.tile([128, 2, 16], FP32, tag="g0f")
            nc.gpsimd.memset(g0f[:], 0.0)
            for kp in range(KP):
                gk = gp.tile([128, 2, 16], FP8, tag=f"g{kp}", name=f"g{kp}")
                nc.vector.tensor_copy(gk[:], g0f[:])
                G8.append(gk)
            hn_prev = []
            for h in range(2):
                hz = hnp.tile([BS, 512], FP32, tag=f"hn{h}", name=f"hnz{h}")
                nc.gpsimd.memset(hz[:], 0.0)
                hn_prev.append(hz)
            with (
                tc.tile_pool(name="mm", bufs=1, space="PSUM") as mmp,
                tc.tile_pool(name="tr", bufs=4, space="PSUM") as trpp,
            ):
                zoct = None
                KORD = (2, 3, 0, 1)  # contraction order == G8 arrival order
                for t in range(t_steps):
                    o, sl = divmod(t, 8)
                    if sl == 0:
                        zoct = ztp.tile([128, HID], FP32R, name="zoct")
                        nc.sync.dma_start(zoct[:], Zd[o])
                    hw = [
                        mmp.tile([BS, 512], FP32, tag=f"hw{h}", name=f"hw{h}")
                        for h in range(2)
                    ]
                    hp = [
                        mmp.tile([BS, 512], FP32, tag=f"hp{h}", name=f"hp{h}")
                        for h in range(2)
                    ]
                    s = [
                        spp.tile([BS, 512], FP32, tag=f"s{h}", name=f"s{h}")
                        for h in range(2)
                    ]
                    v = [
                        up.tile([BS, 512], FP32, tag=f"u{h}", name=f"v{h}")
                        for h in range(2)
                    ]
                    hn = [
                        hnp.tile([BS, 512], FP32, tag=f"hn{h}", name=f"hn{h}")
                        for h in range(2)
                    ]
                    # z/C8 injected into each W-half psum first (no G dep)
                    for h in (1, 0):
                        nc.tensor.matmul(
                            hw[h][:],
                            identR[:, 16 * sl : 16 * sl + 16],
                            zoct[:, 512 * h : 512 * h + 512],
                            start=True,
                            stop=False,
                        )
                    # W-streams, halves interleaved (h1 first), kp in KORD
                    for i, kp in enumerate(KORD):
                        for h in (1, 0):
                            nc.tensor.matmul(
                                hw[h][:],
                                G8[kp][:],
                                W8[:, kp, :, 512 * h : 512 * h + 512],
                                start=False,
                                stop=(i == 3),
                                perf_mode=DR,
                            )
                    for h in (1, 0):
                        # s = tanh(C8 * psum), then v = s + H (Pool add runs
                        # during the A-streams, keeping the tail to one stt)
                        nc.scalar.activation(s[h][:], hw[h][:], AF.Tanh, scale=C8)
                        nc.gpsimd.tensor_add(v[h][:], s[h][:], hn_prev[h][:])
                    # A-streams: h1 group then h0 group, kp in KORD
                    for h in (1, 0):
                        for i, kp in enumerate(KORD):
                            nc.tensor.matmul(
                                hp[h][:],
                                G8[kp][:],
                                A8[:, kp, :, 512 * h : 512 * h + 512],
                                start=(i == 0),
                                stop=(i == 3),
                                perf_mode=DR,
                            )
                    # tails: H' = C8*(H@A) + (tanh + H), then retranspose
                    Gn = [None] * KP
                    for h in (1, 0):
                        nc.vector.scalar_tensor_tensor(
                            hn[h][:], hp[h][:], C8, v[h][:], ALU.mult, ALU.add
                        )
                        for kk in range(2):
                            kp = 2 * h + kk
                            trj = trpp.tile(
                                [128, 32], FP32, tag="trp", name=f"trp{kp}"
                            )
                            for r in range(2):
                                jj = 2 * kk + r
                                nc.tensor.transpose(
                                    trj[:, 16 * r : 16 * r + 16],
                                    hn[h][:, 128 * jj : 128 * jj + 128],
                                    ident[:BS, :BS],
                                )
                            gk = gp.tile(
                                [128, 2, 16], FP8, tag=f"g{kp}", name=f"g{kp}"
                            )
                            nc.vector.tensor_copy(gk[:], trj[:])
                            Gn[kp] = gk
                    G8 = Gn
                    hn_prev = hn

            # ---- final linear: out = dt * (H_T @ D_w.T) + D_b (fp32r) ----
            with (
                tc.tile_pool(name="fin", bufs=1) as fin,
                tc.tile_pool(name="fps", bufs=2, space="PSUM") as fps,
            ):
                Gf = []
                for j in range(KT):
                    h, jj = divmod(j, 4)
                    trj = fps.tile([128, 16], FP32, tag="ftr", name=f"ftr{j}")
                    nc.tensor.transpose(
                        trj[:],
                        hn_prev[h][:, 128 * jj : 128 * jj + 128],
                        ident[:BS, :BS],
                    )
                    gf = fin.tile([128, 16], FP32R, tag=f"gf{j}", name=f"gf{j}")
                    nc.vector.tensor_copy(gf[:], trj[:])
                    Gf.append(gf)
                po = fps.tile([BS, OUT], FP32, tag="po")
                for k in range(KT):
                    nc.tensor.matmul(
                        po[:],
                        Gf[k][:],
                        Dwt_r[:, OUT * k : OUT * k + OUT],
                        start=(k == 0),
                        stop=(k == KT - 1),
                    )
                ob = fin.tile([BS, OUT], FP32)
                nc.vector.scalar_tensor_tensor(
                    ob[:], po[:], DT, Dbb_sb[:BS, :], ALU.mult, ALU.add
                )
                nc.sync.dma_start(out[:], ob[:])

    nc.finalize()
    return nc


def make_in_maps(x, M_W, M_A, E_w, E_b, D_w, D_b):
    f32 = lambda a: np.ascontiguousarray(np.asarray(a, dtype=np.float32))
    x = f32(x)
    Ewt = f32(np.asarray(E_w, np.float32).T)
    Ebb = f32(np.tile(np.asarray(E_b, np.float32)[None, :], (B, 1)))
    Dwt = f32(np.asarray(D_w, np.float32).T)
    Dbb = f32(np.tile(np.asarray(D_b, np.float32)[None, :], (B, 1)))
    MAc, MWc = f32(M_A), f32(M_W)
    in_maps = []
    for c in range(NCORES):
        in_maps.append(
            {
                "xs": f32(x[:, :, BS * c : BS * (c + 1)]),
                "MA": MAc,
                "MW": MWc,
                "Ewt": Ewt,
                "Ebb": Ebb,
                "Dwt": Dwt,
                "Dbb": Dbb,
            }
        )
    return in_maps


_NC_CACHE = {}


def _get_nc(t_steps=T):
    if t_steps not in _NC_CACHE:
        _NC_CACHE[t_steps] = build(t_steps)
    return _NC_CACHE[t_steps]


def kernel(x, M_W, M_A, E_w, E_b, D_w, D_b):
    nc = _get_nc(T)
    in_maps = make_in_maps(x, M_W, M_A, E_w, E_b, D_w, D_b)
    res = run_bass_kernel_spmd(nc, in_maps, list(range(NCORES)))
    return np.concatenate(
        [res.results[c]["out"] for c in range(NCORES)], axis=0
    ).astype(np.float32)

